# revision 21
# baseline (speedup 1.0000x reference)
"""GQA attention kernel for Trainium2, 8-core tensor-parallel over kv heads.

Reference computation (fp32):
  q  = query @ q_proj.T + q_bias      -> heads (g-major): dq = gi*H*D + hi*D + d
  kv = query @ kv_proj.T + kv_bias    -> per kv head hi: k = cols [hi*2D, hi*2D+D), v = next D
  attn = softmax(q k^T / sqrt(D));  out = (attn v) @ out_proj.T + out_bias

Sharding: 8 cores; core c handles kv head h0 = c//2 and 4 query-head groups
gis = [0..3] (c even) or [4..7] (c odd). Each core computes a full-shape
partial of the output (rank-256 contribution); host sums the 8 partials.

On-core dataflow (all matmuls in float32r; t = n*L + l):
  P1: QT[dq,t] = qpT.T @ queryT ; KVT[128,t] (k rows 0:64, v rows 64:128)
      KTdup[64:128] <- K (DMA shift)  ; V'[t,65] via PE-transpose of VT + ones col
  P2: per (n, head-pair, l-half):   scores^T[m,l] = K^T q  (row-tiled 2 heads)
      exp via ACT (scale=1/8 fused) ; AV with ones-augmented V' -> [attnout^T; denom]
      normalize via reciprocal + partition-broadcast DMA + DVE mult
  P3: out_partial[t,e] = attnoutT.T @ opT  (contract local c, 2 chunks of 128)
"""
import sys

sys.path.insert(0, "/opt/trn_rl_repo")

import ml_dtypes
import numpy as np

import concourse.bass as bass
import concourse.mybir as mybir
import concourse.tile as tile
from concourse import bacc

H, G, D = 4, 8, 64
L, N, E = 2048, 2, 2048
T = N * L
P = 128
DQ = 256  # per-core q dim: 4 groups x 64
SCALE = float(D) ** -0.5
F32 = mybir.dt.float32
F32R = mybir.dt.float32r
BF16 = mybir.dt.bfloat16


def r(ap):
    return ap.bitcast(F32R)


def pbcast(ap2d, p):
    """[1, F] AP -> [p, F] AP broadcast across partitions (stride 0)."""
    return bass.AP(tensor=ap2d.tensor, offset=ap2d.offset, ap=[[0, p]] + list(ap2d.ap[1:]))


_LDW_PATCHED = False


def _enable_ldw_opt():
    """walrus dedupes back-to-back identical LDWEIGHTS only with
    --enable-ldw-opt=true; bass_utils hardcodes false. Rewrite the flag."""
    global _LDW_PATCHED
    if _LDW_PATCHED:
        return
    _LDW_PATCHED = True  # ldw-opt=true fails walrus codegen (visitInstLdweights); keep default


def build_nc():
    _enable_ldw_opt()
    nc = bacc.Bacc("TRN2", target_bir_lowering=False, debug=False)
    add = mybir.AluOpType.add

    qT = nc.dram_tensor("qT", [E, T], BF16, kind="ExternalInput").ap()
    qpT = nc.dram_tensor("qpT", [E, DQ], BF16, kind="ExternalInput").ap()
    kvpT = nc.dram_tensor("kvpT", [E, P], BF16, kind="ExternalInput").ap()
    opT = nc.dram_tensor("opT", [DQ, E], BF16, kind="ExternalInput").ap()
    qb = nc.dram_tensor("qb", [P, 2], F32, kind="ExternalInput").ap()
    kvb = nc.dram_tensor("kvb", [P, 1], F32, kind="ExternalInput").ap()
    ident = nc.dram_tensor("ident", [P, P], BF16, kind="ExternalInput").ap()
    ones16 = nc.dram_tensor("ones16", [P, 16], BF16, kind="ExternalInput").ap()
    out = nc.dram_tensor("out", [T, E], F32, kind="ExternalOutput").ap()
    denombuf = nc.dram_tensor("denombuf", [1, 8 * 2048], F32, kind="Internal").ap()
    recipbuf = nc.dram_tensor("recipbuf", [1, 8 * 2048], F32, kind="Internal").ap()

    with tile.TileContext(nc) as tc, tc.tile_pool(name="data", bufs=1) as data, \
            tc.tile_pool(name="consts", bufs=1) as consts:
        identb = consts.tile([P, P], BF16)
        nc.sync.dma_start(out=identb[:], in_=ident)
        qb_sb = consts.tile([P, 2], F32)
        nc.sync.dma_start(out=qb_sb[:], in_=qb)
        kvb_sb = consts.tile([P, 1], F32)
        nc.sync.dma_start(out=kvb_sb[:], in_=kvb)

        QT0 = data.tile([P, T], BF16)  # dq 0:128   (gi_loc 0, 1)
        QT1 = data.tile([P, T], BF16)  # dq 128:256 (gi_loc 2, 3)
        KVT = data.tile([P, T], BF16)  # k rows 0:64, v rows 64:128
        KTdup = data.tile([P, T], BF16)  # k rows duplicated at partitions 64:128
        attn0 = data.tile([P, T], BF16)  # attnoutT c-chunk 0 (gi_loc 0, 1)
        attn1 = data.tile([P, T], BF16)  # c-chunk 1 (gi_loc 2, 3)
        Vp = [data.tile([P, 16 * 65], BF16, name=f"vp{n}", tag=f"vp{n}") for n in range(N)]
        opT_sb = [data.tile([P, E], BF16, name=f"opt{cc}", tag=f"opt{cc}") for cc in range(2)]
        for cc in range(2):
            nc.sync.dma_start(out=opT_sb[cc][:], in_=opT[cc * P:(cc + 1) * P, :])

        # ---------------- Phase 1: projections ----------------
        with tc.tile_pool(name="w1", bufs=1) as w1, \
                tc.tile_pool(name="qload", bufs=6) as qload, \
                tc.tile_pool(name="ps1", bufs=1, space="PSUM") as ps1:
            qpT_sb = w1.tile([P, 16 * DQ], BF16)
            kvpT_sb = w1.tile([P, 16 * P], BF16)
            for ec in range(16):
                nc.sync.dma_start(out=qpT_sb[:, ec * DQ:(ec + 1) * DQ],
                                  in_=qpT[ec * P:(ec + 1) * P, :])
                nc.sync.dma_start(out=kvpT_sb[:, ec * P:(ec + 1) * P],
                                  in_=kvpT[ec * P:(ec + 1) * P, :])
            for tchunk in range(4):
                tcols = slice(tchunk * 1024, (tchunk + 1) * 1024)
                pq0 = ps1.tile([P, 1024], F32, tag="pq0")
                pq1 = ps1.tile([P, 1024], F32, tag="pq1")
                pkv = ps1.tile([P, 1024], F32, tag="pkv")
                for ec in range(16):
                    qt = qload.tile([P, 1024], BF16, tag="qt")
                    nc.sync.dma_start(out=qt[:], in_=qT[ec * P:(ec + 1) * P, tcols])
                    first, last = ec == 0, ec == 15
                    for ps_t, w in ((pq0, qpT_sb[:, ec * DQ:ec * DQ + P]),
                                    (pq1, qpT_sb[:, ec * DQ + P:(ec + 1) * DQ]),
                                    (pkv, kvpT_sb[:, ec * P:(ec + 1) * P])):
                        for lq in range(2):
                            nc.tensor.matmul(ps_t[:, lq * 512:(lq + 1) * 512], lhsT=w,
                                             rhs=qt[:, lq * 512:(lq + 1) * 512],
                                             start=first, stop=last)
                nc.vector.tensor_scalar(QT0[:, tcols], pq0[:], qb_sb[:, 0:1], None, op0=add)
                nc.scalar.activation(QT1[:, tcols], pq1[:],
                                     mybir.ActivationFunctionType.Identity,
                                     bias=qb_sb[:, 1:2])
                nc.vector.tensor_scalar(KVT[:, tcols], pkv[:], kvb_sb[:, 0:1], None, op0=add)
                nc.sync.dma_start(out=KTdup[64:128, tcols], in_=KVT[0:64, tcols])

        # V' = transpose(VT) + ones column
        with tc.tile_pool(name="vtmp", bufs=1) as vtmp_pool, \
                tc.tile_pool(name="psT", bufs=4, space="PSUM") as psT:
            Vtmp = vtmp_pool.tile([64, T], BF16)
            nc.sync.dma_start(out=Vtmp[:], in_=KVT[64:128, :])
            for n in range(N):
                vcol = Vp[n].rearrange("p (m c) -> p m c", c=65)[:, :, 64:65]
                nc.sync.dma_start(out=vcol, in_=ones16)
                for mc in range(16):
                    pt = psT.tile([P, 64], BF16, tag="pt")
                    nc.tensor.transpose(pt[:], Vtmp[0:64, n * L + mc * P:n * L + (mc + 1) * P],
                                        identb[0:64, 0:64])
                    nc.vector.tensor_copy(Vp[n][:, mc * 65:mc * 65 + 64], pt[:])

        # ---------------- Phase 2: attention ----------------
        with tc.tile_pool(name="ps_s", bufs=1, space="PSUM") as ps_s, \
                tc.tile_pool(name="ps_av", bufs=1, space="PSUM") as ps_av, \
                tc.tile_pool(name="expool", bufs=3) as expool, \
                tc.tile_pool(name="scratch", bufs=2) as scratch:
            for n in range(N):
                for pair in range(2):
                    QTp = QT0 if pair == 0 else QT1
                    attnp = attn0 if pair == 0 else attn1
                    for half in range(2):
                        lo = n * L + half * 1024
                        lcols = slice(lo, lo + 1024)
                        avA = ps_av.tile([65, 1024], F32, tag="avA")
                        avB = ps_av.tile([65, 1024], F32, tag="avB")
                        for mc in range(16):
                            mo = n * L + mc * P
                            sA = ps_s.tile([P, 1024], F32, tag="sA")
                            sB = ps_s.tile([P, 1024], F32, tag="sB")
                            for lq in range(2):
                                lc2 = slice(lo + lq * 512, lo + lq * 512 + 512)
                                nc.tensor.matmul(sA[:, lq * 512:(lq + 1) * 512],
                                                 lhsT=KVT[0:64, mo:mo + P],
                                                 rhs=QTp[0:64, lc2])
                                nc.tensor.matmul(sB[:, lq * 512:(lq + 1) * 512],
                                                 lhsT=KTdup[64:128, mo:mo + P],
                                                 rhs=QTp[64:128, lc2])
                            eA = expool.tile([P, 1024], BF16, tag="eA")
                            eB = expool.tile([P, 1024], BF16, tag="eB")
                            nc.scalar.activation(eA[:], sA[:], mybir.ActivationFunctionType.Exp,
                                                 scale=SCALE)
                            nc.scalar.activation(eB[:], sB[:], mybir.ActivationFunctionType.Exp,
                                                 scale=SCALE)
                            first, last = mc == 0, mc == 15
                            vw = Vp[n][:, mc * 65:mc * 65 + 65]
                            for lq in range(2):
                                nc.tensor.matmul(avA[:, lq * 512:(lq + 1) * 512], lhsT=vw,
                                                 rhs=eA[:, lq * 512:(lq + 1) * 512],
                                                 start=first, stop=last)
                            for lq in range(2):
                                nc.tensor.matmul(avB[:, lq * 512:(lq + 1) * 512], lhsT=vw,
                                                 rhs=eB[:, lq * 512:(lq + 1) * 512],
                                                 start=first, stop=last)
                        # evict unnormalized attnout rows + denominator rows
                        nc.vector.tensor_copy(attnp[0:64, lcols], avA[0:64, :])
                        sc = scratch.tile([64, 1024], BF16, tag="sc")
                        nc.vector.tensor_copy(sc[:], avB[0:64, :])
                        nc.sync.dma_start(out=attnp[64:128, lcols], in_=sc[:])
                        dA = (n * 4 + 2 * pair) * 2048 + half * 1024
                        dB = (n * 4 + 2 * pair + 1) * 2048 + half * 1024
                        dnA = scratch.tile([1, 1024], F32, tag="dnA")
                        dnB = scratch.tile([1, 1024], F32, tag="dnB")
                        nc.vector.tensor_copy(dnA[:], avA[64:65, :])
                        nc.vector.tensor_copy(dnB[:], avB[64:65, :])
                        nc.sync.dma_start(out=denombuf[0:1, dA:dA + 1024], in_=dnA[:])
                        nc.sync.dma_start(out=denombuf[0:1, dB:dB + 1024], in_=dnB[:])
                    # normalize this (n, pair) immediately; overlaps next pair's attention
                    seg = (n * 4 + 2 * pair) * 2048
                    packed = scratch.tile([P, 32], F32, tag="packed")
                    nc.sync.dma_start(
                        out=packed[:],
                        in_=denombuf[0:1, seg:seg + 4096].rearrange("a (p c) -> (a p) c", p=P))
                    recp = scratch.tile([P, 32], F32, tag="recp")
                    nc.vector.reciprocal(recp[:], packed[:])
                    nc.sync.dma_start(
                        out=recipbuf[0:1, seg:seg + 4096].rearrange("a (p c) -> (a p) c", p=P),
                        in_=recp[:])
                    bct = scratch.tile([P, L], F32, tag="bct")
                    nc.sync.dma_start(out=bct[0:64, :],
                                      in_=pbcast(recipbuf[0:1, seg:seg + 2048], 64))
                    nc.sync.dma_start(out=bct[64:128, :],
                                      in_=pbcast(recipbuf[0:1, seg + 2048:seg + 4096], 64))
                    ncols = slice(n * L, (n + 1) * L)
                    nc.vector.tensor_mul(attnp[:, ncols], attnp[:, ncols], bct[:])

        # ---------------- Phase 3: output projection ----------------
        with tc.tile_pool(name="psO", bufs=4, space="PSUM") as psO, \
                tc.tile_pool(name="ostage", bufs=3) as ostage:
            for tt in range(32):
                trows = slice(tt * P, (tt + 1) * P)
                po = [psO.tile([P, 1024], F32, name="po", tag="po") for _ in range(2)]
                for cc in range(2):
                    src = attn0 if cc == 0 else attn1
                    for eo in range(2):
                        for lq in range(2):
                            nc.tensor.matmul(po[eo][:, lq * 512:(lq + 1) * 512],
                                             lhsT=src[:, trows],
                                             rhs=opT_sb[cc][:, eo * 1024 + lq * 512:
                                                            eo * 1024 + (lq + 1) * 512],
                                             start=cc == 0, stop=cc == 1)
                ost = ostage.tile([P, E], F32, tag="ost")
                nc.vector.tensor_copy(ost[:, 0:1024], po[0][:])
                nc.scalar.copy(ost[:, 1024:2048], po[1][:])
                nc.sync.dma_start(out=out[trows, :], in_=ost[:])

    nc.compile()
    return nc


_NC_CACHE = None


def _get_nc():
    global _NC_CACHE
    if _NC_CACHE is None:
        _NC_CACHE = build_nc()
    return _NC_CACHE


def make_in_maps(query, q_proj, q_bias, kv_proj, kv_bias, out_proj):
    """Host-side sharding. Returns list of 8 per-core input dicts."""
    qT_h = np.ascontiguousarray(
        np.asarray(query, dtype=np.float32).transpose(2, 1, 0).reshape(E, T)
    ).astype(ml_dtypes.bfloat16)
    q_proj = np.asarray(q_proj, dtype=np.float32)
    q_bias = np.asarray(q_bias, dtype=np.float32)
    kv_proj = np.asarray(kv_proj, dtype=np.float32)
    kv_bias = np.asarray(kv_bias, dtype=np.float32)
    out_proj = np.asarray(out_proj, dtype=np.float32)
    ident = np.eye(P, dtype=np.float32)

    in_maps = []
    for c in range(8):
        h0 = c // 2
        gis = range(4) if c % 2 == 0 else range(4, 8)
        rows_q = np.array([gi * (H * D) + h0 * D + d for gi in gis for d in range(D)])
        kv_rows = slice(h0 * 2 * D, (h0 + 1) * 2 * D)
        in_maps.append({
            "qT": qT_h,
            "qpT": np.ascontiguousarray(q_proj[rows_q, :].T).astype(ml_dtypes.bfloat16),
            "kvpT": np.ascontiguousarray(kv_proj[kv_rows, :].T).astype(ml_dtypes.bfloat16),
            "opT": np.ascontiguousarray(out_proj[:, rows_q].T).astype(ml_dtypes.bfloat16),
            "qb": np.ascontiguousarray(q_bias[rows_q].reshape(2, P).T),
            "kvb": np.ascontiguousarray(kv_bias[kv_rows].reshape(P, 1)),
            "ident": ident.astype(ml_dtypes.bfloat16),
            "ones16": np.ones((P, 16), dtype=ml_dtypes.bfloat16),
        })
    return in_maps


def kernel(query, q_proj, q_bias, kv_proj, kv_bias, out_proj, out_bias):
    from concourse.bass_utils import run_bass_kernel_spmd

    nc = _get_nc()
    in_maps = make_in_maps(query, q_proj, q_bias, kv_proj, kv_bias, out_proj)
    res = run_bass_kernel_spmd(nc, in_maps, core_ids=list(range(8)))
    total = np.zeros((T, E), dtype=np.float64)
    for rmap in res.results:
        total += rmap["out"].astype(np.float64)
    total += np.asarray(out_bias, dtype=np.float64)[None, :]
    return np.ascontiguousarray(
        total.reshape(N, L, E).transpose(1, 0, 2)).astype(np.float32)


# revision 22
# speedup vs baseline: 1.1271x; 1.1271x over previous
"""GQA attention kernel for Trainium2, 8-core tensor-parallel over kv heads.

Reference computation (fp32):
  q  = query @ q_proj.T + q_bias      -> heads (g-major): dq = gi*H*D + hi*D + d
  kv = query @ kv_proj.T + kv_bias    -> per kv head hi: k = cols [hi*2D, hi*2D+D), v = next D
  attn = softmax(q k^T / sqrt(D));  out = (attn v) @ out_proj.T + out_bias

Sharding: 8 cores; core c handles kv head h0 = c//2 and 4 query-head groups
gis = [0..3] (c even) or [4..7] (c odd). Each core computes a full-shape
partial of the output (rank-256 contribution); host sums the 8 partials.

On-core dataflow (all matmuls in float32r; t = n*L + l):
  P1: QT[dq,t] = qpT.T @ queryT ; KVT[128,t] (k rows 0:64, v rows 64:128)
      KTdup[64:128] <- K (DMA shift)  ; V'[t,65] via PE-transpose of VT + ones col
  P2: per (n, head-pair, l-half):   scores^T[m,l] = K^T q  (row-tiled 2 heads)
      exp via ACT (scale=1/8 fused) ; AV with ones-augmented V' -> [attnout^T; denom]
      normalize via reciprocal + partition-broadcast DMA + DVE mult
  P3: out_partial[t,e] = attnoutT.T @ opT  (contract local c, 2 chunks of 128)
"""
import sys

sys.path.insert(0, "/opt/trn_rl_repo")

import ml_dtypes
import numpy as np

import concourse.bass as bass
import concourse.mybir as mybir
import concourse.tile as tile
from concourse import bacc

H, G, D = 4, 8, 64
L, N, E = 2048, 2, 2048
T = N * L
P = 128
DQ = 256  # per-core q dim: 4 groups x 64
SCALE = float(D) ** -0.5
F32 = mybir.dt.float32
F32R = mybir.dt.float32r
BF16 = mybir.dt.bfloat16


def r(ap):
    return ap.bitcast(F32R)


def pbcast(ap2d, p):
    """[1, F] AP -> [p, F] AP broadcast across partitions (stride 0)."""
    return bass.AP(tensor=ap2d.tensor, offset=ap2d.offset, ap=[[0, p]] + list(ap2d.ap[1:]))


_LDW_PATCHED = False


def _enable_ldw_opt():
    """walrus dedupes back-to-back identical LDWEIGHTS only with
    --enable-ldw-opt=true; bass_utils hardcodes false. Rewrite the flag."""
    global _LDW_PATCHED
    if _LDW_PATCHED:
        return
    _LDW_PATCHED = True  # ldw-opt=true fails walrus codegen (visitInstLdweights); keep default


def build_nc():
    _enable_ldw_opt()
    nc = bacc.Bacc("TRN2", target_bir_lowering=False, debug=False)
    add = mybir.AluOpType.add

    qT = nc.dram_tensor("qT", [E, T], BF16, kind="ExternalInput").ap()
    qpT = nc.dram_tensor("qpT", [E, DQ], BF16, kind="ExternalInput").ap()
    kvpT = nc.dram_tensor("kvpT", [E, P], BF16, kind="ExternalInput").ap()
    opT = nc.dram_tensor("opT", [DQ, E], BF16, kind="ExternalInput").ap()
    qb = nc.dram_tensor("qb", [P, 2], F32, kind="ExternalInput").ap()
    kvb = nc.dram_tensor("kvb", [P, 1], F32, kind="ExternalInput").ap()
    ident = nc.dram_tensor("ident", [P, P], BF16, kind="ExternalInput").ap()
    ones16 = nc.dram_tensor("ones16", [P, 16], BF16, kind="ExternalInput").ap()
    out = nc.dram_tensor("out", [T, E], F32, kind="ExternalOutput").ap()
    denombuf = nc.dram_tensor("denombuf", [1, 8 * 2048], F32, kind="Internal").ap()
    recipbuf = nc.dram_tensor("recipbuf", [1, 8 * 2048], F32, kind="Internal").ap()

    with tile.TileContext(nc) as tc, tc.tile_pool(name="data", bufs=1) as data, \
            tc.tile_pool(name="consts", bufs=1) as consts:
        identb = consts.tile([P, P], BF16)
        nc.sync.dma_start(out=identb[:], in_=ident)
        qb_sb = consts.tile([P, 2], F32)
        nc.sync.dma_start(out=qb_sb[:], in_=qb)
        kvb_sb = consts.tile([P, 1], F32)
        nc.sync.dma_start(out=kvb_sb[:], in_=kvb)

        QT0 = data.tile([P, T], BF16)  # dq 0:128   (gi_loc 0, 1)
        QT1 = data.tile([P, T], BF16)  # dq 128:256 (gi_loc 2, 3)
        KVT = data.tile([P, T], BF16)  # k rows 0:64, v rows 64:128
        KTdup = data.tile([P, T], BF16)  # k rows duplicated at partitions 64:128
        attn0 = data.tile([P, T], BF16)  # attnoutT c-chunk 0 (gi_loc 0, 1)
        attn1 = data.tile([P, T], BF16)  # c-chunk 1 (gi_loc 2, 3)
        Vp = [data.tile([P, 16 * 65], BF16, name=f"vp{n}", tag=f"vp{n}") for n in range(N)]
        opT_sb = [data.tile([P, E], BF16, name=f"opt{cc}", tag=f"opt{cc}") for cc in range(2)]
        for cc in range(2):
            nc.sync.dma_start(out=opT_sb[cc][:], in_=opT[cc * P:(cc + 1) * P, :])

        # ---------------- Phase 1: projections ----------------
        with tc.tile_pool(name="w1", bufs=1) as w1, \
                tc.tile_pool(name="qload", bufs=6) as qload, \
                tc.tile_pool(name="ps1", bufs=1, space="PSUM") as ps1:
            qpT_sb = w1.tile([P, 16 * DQ], BF16)
            kvpT_sb = w1.tile([P, 16 * P], BF16)
            for ec in range(16):
                nc.sync.dma_start(out=qpT_sb[:, ec * DQ:(ec + 1) * DQ],
                                  in_=qpT[ec * P:(ec + 1) * P, :])
                nc.sync.dma_start(out=kvpT_sb[:, ec * P:(ec + 1) * P],
                                  in_=kvpT[ec * P:(ec + 1) * P, :])
            for tchunk in range(4):
                tcols = slice(tchunk * 1024, (tchunk + 1) * 1024)
                pq0 = ps1.tile([P, 1024], F32, tag="pq0")
                pq1 = ps1.tile([P, 1024], F32, tag="pq1")
                pkv = ps1.tile([P, 1024], F32, tag="pkv")
                for ec in range(16):
                    qt = qload.tile([P, 1024], BF16, tag="qt")
                    nc.sync.dma_start(out=qt[:], in_=qT[ec * P:(ec + 1) * P, tcols])
                    first, last = ec == 0, ec == 15
                    for ps_t, w in ((pq0, qpT_sb[:, ec * DQ:ec * DQ + P]),
                                    (pq1, qpT_sb[:, ec * DQ + P:(ec + 1) * DQ]),
                                    (pkv, kvpT_sb[:, ec * P:(ec + 1) * P])):
                        for lq in range(2):
                            nc.tensor.matmul(ps_t[:, lq * 512:(lq + 1) * 512], lhsT=w,
                                             rhs=qt[:, lq * 512:(lq + 1) * 512],
                                             start=first, stop=last)
                nc.vector.tensor_scalar(QT0[:, tcols], pq0[:], qb_sb[:, 0:1], None, op0=add)
                nc.scalar.activation(QT1[:, tcols], pq1[:],
                                     mybir.ActivationFunctionType.Identity,
                                     bias=qb_sb[:, 1:2])
                nc.vector.tensor_scalar(KVT[:, tcols], pkv[:], kvb_sb[:, 0:1], None, op0=add)
                nc.sync.dma_start(out=KTdup[64:128, tcols], in_=KVT[0:64, tcols])

        # V' = transpose(VT) + ones column
        with tc.tile_pool(name="vtmp", bufs=1) as vtmp_pool, \
                tc.tile_pool(name="psT", bufs=4, space="PSUM") as psT:
            Vtmp = vtmp_pool.tile([64, T], BF16)
            nc.sync.dma_start(out=Vtmp[:], in_=KVT[64:128, :])
            for n in range(N):
                vcol = Vp[n].rearrange("p (m c) -> p m c", c=65)[:, :, 64:65]
                nc.sync.dma_start(out=vcol, in_=ones16)
                for mc in range(16):
                    pt = psT.tile([P, 64], BF16, tag="pt")
                    nc.tensor.transpose(pt[:], Vtmp[0:64, n * L + mc * P:n * L + (mc + 1) * P],
                                        identb[0:64, 0:64])
                    nc.vector.tensor_copy(Vp[n][:, mc * 65:mc * 65 + 64], pt[:])

        # ---------------- Phase 2: attention ----------------
        with tc.tile_pool(name="ps_s", bufs=1, space="PSUM") as ps_s, \
                tc.tile_pool(name="ps_av", bufs=1, space="PSUM") as ps_av, \
                tc.tile_pool(name="expool", bufs=3) as expool, \
                tc.tile_pool(name="scratch", bufs=2) as scratch:
            for n in range(N):
                for pair in range(2):
                    QTp = QT0 if pair == 0 else QT1
                    attnp = attn0 if pair == 0 else attn1
                    for half in range(2):
                        lo = n * L + half * 1024
                        lcols = slice(lo, lo + 1024)
                        avA = ps_av.tile([65, 1024], F32, tag="avA")
                        avB = ps_av.tile([65, 1024], F32, tag="avB")
                        for mc in range(16):
                            mo = n * L + mc * P
                            sA = ps_s.tile([P, 1024], F32, tag="sA")
                            sB = ps_s.tile([P, 1024], F32, tag="sB")
                            for lq in range(2):
                                lc2 = slice(lo + lq * 512, lo + lq * 512 + 512)
                                nc.tensor.matmul(sA[:, lq * 512:(lq + 1) * 512],
                                                 lhsT=KVT[0:64, mo:mo + P],
                                                 rhs=QTp[0:64, lc2])
                            for lq in range(2):
                                lc2 = slice(lo + lq * 512, lo + lq * 512 + 512)
                                nc.tensor.matmul(sB[:, lq * 512:(lq + 1) * 512],
                                                 lhsT=KTdup[64:128, mo:mo + P],
                                                 rhs=QTp[64:128, lc2])
                            eA = expool.tile([P, 1024], BF16, tag="eA")
                            eB = expool.tile([P, 1024], BF16, tag="eB")
                            nc.scalar.activation(eA[:], sA[:], mybir.ActivationFunctionType.Exp,
                                                 scale=SCALE)
                            nc.scalar.activation(eB[:], sB[:], mybir.ActivationFunctionType.Exp,
                                                 scale=SCALE)
                            first, last = mc == 0, mc == 15
                            vw = Vp[n][:, mc * 65:mc * 65 + 65]
                            for lq in range(2):
                                nc.tensor.matmul(avA[:, lq * 512:(lq + 1) * 512], lhsT=vw,
                                                 rhs=eA[:, lq * 512:(lq + 1) * 512],
                                                 start=first, stop=last)
                            for lq in range(2):
                                nc.tensor.matmul(avB[:, lq * 512:(lq + 1) * 512], lhsT=vw,
                                                 rhs=eB[:, lq * 512:(lq + 1) * 512],
                                                 start=first, stop=last)
                        # evict unnormalized attnout rows + denominator rows
                        nc.vector.tensor_copy(attnp[0:64, lcols], avA[0:64, :])
                        sc = scratch.tile([64, 1024], BF16, tag="sc")
                        nc.vector.tensor_copy(sc[:], avB[0:64, :])
                        nc.sync.dma_start(out=attnp[64:128, lcols], in_=sc[:])
                        dA = (n * 4 + 2 * pair) * 2048 + half * 1024
                        dB = (n * 4 + 2 * pair + 1) * 2048 + half * 1024
                        dnA = scratch.tile([1, 1024], F32, tag="dnA")
                        dnB = scratch.tile([1, 1024], F32, tag="dnB")
                        nc.vector.tensor_copy(dnA[:], avA[64:65, :])
                        nc.vector.tensor_copy(dnB[:], avB[64:65, :])
                        nc.sync.dma_start(out=denombuf[0:1, dA:dA + 1024], in_=dnA[:])
                        nc.sync.dma_start(out=denombuf[0:1, dB:dB + 1024], in_=dnB[:])
                    # normalize this (n, pair) immediately; overlaps next pair's attention
                    seg = (n * 4 + 2 * pair) * 2048
                    packed = scratch.tile([P, 32], F32, tag="packed")
                    nc.sync.dma_start(
                        out=packed[:],
                        in_=denombuf[0:1, seg:seg + 4096].rearrange("a (p c) -> (a p) c", p=P))
                    recp = scratch.tile([P, 32], F32, tag="recp")
                    nc.vector.reciprocal(recp[:], packed[:])
                    nc.sync.dma_start(
                        out=recipbuf[0:1, seg:seg + 4096].rearrange("a (p c) -> (a p) c", p=P),
                        in_=recp[:])
                    bct = scratch.tile([P, L], F32, tag="bct")
                    nc.sync.dma_start(out=bct[0:64, :],
                                      in_=pbcast(recipbuf[0:1, seg:seg + 2048], 64))
                    nc.sync.dma_start(out=bct[64:128, :],
                                      in_=pbcast(recipbuf[0:1, seg + 2048:seg + 4096], 64))
                    ncols = slice(n * L, (n + 1) * L)
                    nc.vector.tensor_mul(attnp[:, ncols], attnp[:, ncols], bct[:])

        # ---------------- Phase 3: output projection ----------------
        with tc.tile_pool(name="psO", bufs=4, space="PSUM") as psO, \
                tc.tile_pool(name="ostage", bufs=3) as ostage:
            for tt in range(32):
                trows = slice(tt * P, (tt + 1) * P)
                po = [psO.tile([P, 1024], F32, name="po", tag="po") for _ in range(2)]
                for cc in range(2):
                    src = attn0 if cc == 0 else attn1
                    for eo in range(2):
                        for lq in range(2):
                            nc.tensor.matmul(po[eo][:, lq * 512:(lq + 1) * 512],
                                             lhsT=src[:, trows],
                                             rhs=opT_sb[cc][:, eo * 1024 + lq * 512:
                                                            eo * 1024 + (lq + 1) * 512],
                                             start=cc == 0, stop=cc == 1)
                ost = ostage.tile([P, E], F32, tag="ost")
                nc.vector.tensor_copy(ost[:, 0:1024], po[0][:])
                nc.scalar.copy(ost[:, 1024:2048], po[1][:])
                nc.sync.dma_start(out=out[trows, :], in_=ost[:])

    nc.compile()
    return nc


_NC_CACHE = None


def _get_nc():
    global _NC_CACHE
    if _NC_CACHE is None:
        _NC_CACHE = build_nc()
    return _NC_CACHE


def make_in_maps(query, q_proj, q_bias, kv_proj, kv_bias, out_proj):
    """Host-side sharding. Returns list of 8 per-core input dicts."""
    qT_h = np.ascontiguousarray(
        np.asarray(query, dtype=np.float32).transpose(2, 1, 0).reshape(E, T)
    ).astype(ml_dtypes.bfloat16)
    q_proj = np.asarray(q_proj, dtype=np.float32)
    q_bias = np.asarray(q_bias, dtype=np.float32)
    kv_proj = np.asarray(kv_proj, dtype=np.float32)
    kv_bias = np.asarray(kv_bias, dtype=np.float32)
    out_proj = np.asarray(out_proj, dtype=np.float32)
    ident = np.eye(P, dtype=np.float32)

    in_maps = []
    for c in range(8):
        h0 = c // 2
        gis = range(4) if c % 2 == 0 else range(4, 8)
        rows_q = np.array([gi * (H * D) + h0 * D + d for gi in gis for d in range(D)])
        kv_rows = slice(h0 * 2 * D, (h0 + 1) * 2 * D)
        in_maps.append({
            "qT": qT_h,
            "qpT": np.ascontiguousarray(q_proj[rows_q, :].T).astype(ml_dtypes.bfloat16),
            "kvpT": np.ascontiguousarray(kv_proj[kv_rows, :].T).astype(ml_dtypes.bfloat16),
            "opT": np.ascontiguousarray(out_proj[:, rows_q].T).astype(ml_dtypes.bfloat16),
            "qb": np.ascontiguousarray(q_bias[rows_q].reshape(2, P).T),
            "kvb": np.ascontiguousarray(kv_bias[kv_rows].reshape(P, 1)),
            "ident": ident.astype(ml_dtypes.bfloat16),
            "ones16": np.ones((P, 16), dtype=ml_dtypes.bfloat16),
        })
    return in_maps


def kernel(query, q_proj, q_bias, kv_proj, kv_bias, out_proj, out_bias):
    from concourse.bass_utils import run_bass_kernel_spmd

    nc = _get_nc()
    in_maps = make_in_maps(query, q_proj, q_bias, kv_proj, kv_bias, out_proj)
    res = run_bass_kernel_spmd(nc, in_maps, core_ids=list(range(8)))
    total = np.zeros((T, E), dtype=np.float64)
    for rmap in res.results:
        total += rmap["out"].astype(np.float64)
    total += np.asarray(out_bias, dtype=np.float64)[None, :]
    return np.ascontiguousarray(
        total.reshape(N, L, E).transpose(1, 0, 2)).astype(np.float32)
